# revision 1
# baseline (speedup 1.0000x reference)
"""CoLA GNN model kernel for 8 Trainium2 NeuronCores.

Math (per branch, pos/neg):
  xw   = x @ W_gcn                                   [N, 256]
  agg  = scatter_add(dst, w * xw[src])               [N, 256]
  h    = PReLU(agg + b_gcn)                          [N, 256]
  pool = l2norm(mean(h over nodes 0..6 per subgraph))
  anch = l2norm(h node 7 per subgraph)               (pos branch only)
  score_b = pool_b . (W_bil @ anch_b) + b_bil

Device mapping (per core: 1024 subgraphs = 8192 nodes per branch):
  - edges are subgraph-local; combined index c = 8*(src%8) + (dst%8)
  - A_flat[b, c] histogram built on DVE (compare/mult/add-tree vs expanded iota)
  - A_flat -> block-diagonal BDT tiles via DRAM-staged strided DMAs
  - xw via PE (x transposed on PE, bf16), agg via BDT matmul,
    pooling via h-stationary matmuls producing transposed pool/anchor,
  - l2 normalization deferred into final per-subgraph scalars.
"""

import numpy as np
import ml_dtypes

import concourse.mybir as mybir
import concourse.tile as tile
from concourse import bacc
from concourse.bass_utils import run_bass_kernel_spmd
from concourse.masks import make_identity

F32 = mybir.dt.float32
BF16 = mybir.dt.bfloat16
AX = mybir.AluOpType

N_CORES = 8
S = 8                     # nodes per subgraph
B_TOT = 8192              # subgraphs total
BC = B_TOT // N_CORES     # subgraphs per core (1024)
NC_NODES = BC * S         # nodes per core (8192)
DIN = 512
DOUT = 256
EPB = 64                  # edges per subgraph
NBLK = NC_NODES // 128    # 64 row-blocks of 128 nodes (16 subgraphs) per branch
HT = BC // 128            # histogram tiles per branch (8)
EPS = 1e-12

_KERNEL_CACHE = {}


def _build(use_bias: bool):
    nc = bacc.Bacc(None, target_bir_lowering=False)

    # ---- I/O ----
    x_pos = nc.dram_tensor("x_pos", [NC_NODES, DIN], F32, kind="ExternalInput")
    x_neg = nc.dram_tensor("x_neg", [NC_NODES, DIN], F32, kind="ExternalInput")
    wg_in = nc.dram_tensor("wg_in", [128, 4 * DOUT], BF16, kind="ExternalInput")
    wbt_in = nc.dram_tensor("wbt_in", [128, 512], BF16, kind="ExternalInput")
    pam_in = nc.dram_tensor("pam_in", [128, 32], BF16, kind="ExternalInput")
    iota_in = nc.dram_tensor("iota_in", [128, 4096], BF16, kind="ExternalInput")
    a_in = nc.dram_tensor("a_in", [128, 1], F32, kind="ExternalInput")
    bbil_in = nc.dram_tensor("bbil_in", [128, 1], F32, kind="ExternalInput")
    cidx_pos = nc.dram_tensor("cidx_pos", [128, HT * EPB], BF16, kind="ExternalInput")
    cidx_neg = nc.dram_tensor("cidx_neg", [128, HT * EPB], BF16, kind="ExternalInput")
    ew_pos = nc.dram_tensor("ew_pos", [128, HT * EPB], BF16, kind="ExternalInput")
    ew_neg = nc.dram_tensor("ew_neg", [128, HT * EPB], BF16, kind="ExternalInput")
    if use_bias:
        bgcn_in = nc.dram_tensor("bgcn_in", [1, DOUT], F32, kind="ExternalInput")
    scores_out = nc.dram_tensor("scores_out", [2, BC], F32, kind="ExternalOutput")

    with tile.TileContext(nc) as tc:
        with tc.tile_pool(name="const", bufs=1) as cpool, \
             tc.tile_pool(name="dram", bufs=1, space="DRAM") as dpool, \
             tc.tile_pool(name="persist", bufs=1) as ppool:

            # ---- constants ----
            ident = cpool.tile([128, 128], BF16)
            make_identity(nc, ident)
            wg = cpool.tile([128, 4 * DOUT], BF16)
            nc.sync.dma_start(wg[:], wg_in[:])
            wbt = cpool.tile([128, 512], BF16)
            nc.sync.dma_start(wbt[:], wbt_in[:])
            pam = cpool.tile([128, 32], BF16)
            nc.sync.dma_start(pam[:], pam_in[:])
            iota_e = cpool.tile([128, 4096], BF16)
            nc.sync.dma_start(iota_e[:], iota_in[:])
            a_rep = cpool.tile([128, 1], F32)
            nc.sync.dma_start(a_rep[:], a_in[:])
            bbil = cpool.tile([128, 1], F32)
            nc.sync.dma_start(bbil[:], bbil_in[:])
            ones_col = cpool.tile([128, 1], BF16)
            nc.vector.memset(ones_col[:], 1.0)
            if use_bias:
                bg_row = cpool.tile([1, DOUT], F32)
                nc.sync.dma_start(bg_row[:], bgcn_in[:])
                bg_bc = cpool.tile([128, DOUT], F32)
                nc.gpsimd.partition_broadcast(bg_bc[:], bg_row[:])

            # persistent per-branch state
            bdt = {}
            cidx_sb = {}
            ew_sb = {}
            poolt = {}   # [dc] -> [128, NBLK*32] bf16, transposed pools+anchors
            for br, (ci_in, w_in) in (("pos", (cidx_pos, ew_pos)),
                                      ("neg", (cidx_neg, ew_neg))):
                bdt[br] = ppool.tile([128, NBLK * 128], BF16,
                                     name=f"bdt_{br}", tag=f"bdt_{br}")
                nc.gpsimd.memset(bdt[br][:], 0.0)
                t = ppool.tile([128, HT * EPB], BF16,
                               name=f"cidx_{br}", tag=f"cidx_{br}")
                nc.sync.dma_start(t[:], ci_in[:])
                cidx_sb[br] = t
                t = ppool.tile([128, HT * EPB], BF16,
                               name=f"ew_{br}", tag=f"ew_{br}")
                nc.sync.dma_start(t[:], w_in[:])
                ew_sb[br] = t
                poolt[br] = [
                    ppool.tile([128, NBLK * 32], BF16,
                               name=f"poolt_{br}{dc}", tag=f"poolt_{br}{dc}")
                    for dc in range(2)
                ]

            # ====== fused per-t-group: histogram -> placement -> 8 blocks ======
            XB = 4          # x row-blocks per DMA (1 MiB)
            iota3 = iota_e[:].rearrange("p (c e) -> p c e", e=64)
            with tc.tile_pool(name="hist", bufs=3) as hpool, \
                 tc.tile_pool(name="blk", bufs=6) as bpool, \
                 tc.tile_pool(name="ps", bufs=2, space="PSUM") as pspool, \
                 tc.tile_pool(name="pspool2", bufs=1, space="PSUM") as pqpool:
                for br, x_in in (("pos", x_pos), ("neg", x_neg)):
                    stage = dpool.tile([BC, EPB], BF16,
                                       name=f"stage_{br}", tag=f"stage_{br}")
                    src6 = stage[:].rearrange(
                        "(t bb j) (s d) -> j s t bb d", t=8, bb=8, j=16, d=8)
                    dst6 = bdt[br][:].rearrange(
                        "q (t bb x d) -> q t bb x d", t=8, bb=8, x=16, d=8)
                    ps_pt = None
                    for q in range(2):
                        for t in range(4 * q, 4 * q + 4):
                            ci = cidx_sb[br][:, t * 64:(t + 1) * 64]
                            w3 = ew_sb[br][:, t * 64:(t + 1) * 64] \
                                .unsqueeze(1).broadcast_to((128, 64, 64))
                            ci3 = ci.unsqueeze(1).broadcast_to((128, 64, 64))
                            mask = hpool.tile([128, 4096], BF16, tag="mask")
                            k3 = mask[:].rearrange("p (c e) -> p c e", e=64)
                            nc.vector.tensor_tensor(k3, ci3, iota3, AX.is_equal)
                            masked = hpool.tile([128, 4096], BF16, tag="masked")
                            m3 = masked[:].rearrange("p (c e) -> p c e", e=64)
                            nc.vector.tensor_tensor(m3, k3, w3, AX.mult)
                            for wd in (32, 16, 8, 4, 2, 1):
                                nc.vector.tensor_tensor(
                                    m3[:, :, 0:wd], m3[:, :, 0:wd],
                                    m3[:, :, wd:2 * wd], AX.add)
                            aflat = hpool.tile([128, EPB], BF16, tag="aflat")
                            nc.vector.tensor_copy(aflat[:], m3[:, :, 0])
                            nc.sync.dma_start(stage[128 * t:128 * (t + 1), :], aflat[:])
                        for hh in range(2):
                            tt0 = 4 * q + 2 * hh
                            for j in range(16):
                                nc.sync.dma_start(
                                    dst6[8 * j:8 * j + 8, tt0:tt0 + 2, :, j, :],
                                    src6[j, :, tt0:tt0 + 2, :, :])
                        for t in range(4 * q, 4 * q + 4):
                            for pi in range(4):
                                B0 = 8 * t + 2 * pi
                                if B0 % XB == 0:
                                    xb = bpool.tile([128, XB * DIN], BF16, tag="xb")
                                    nc.gpsimd.dma_start(
                                        xb[:].rearrange("p (v c) -> p v c", v=XB),
                                        x_in[:].rearrange(
                                            "(u p) c -> p u c", p=128)[:, B0:B0 + XB, :])
                                ps_xt = pspool.tile([128, 2 * DIN], BF16, tag="xt")
                                for half in range(2):
                                    xcur = xb[:, ((B0 + half) % XB) * DIN:
                                              ((B0 + half) % XB + 1) * DIN]
                                    for k in range(4):
                                        nc.tensor.transpose(
                                            ps_xt[:, half * DIN + k * 128:
                                                  half * DIN + (k + 1) * 128],
                                            xcur[:, k * 128:(k + 1) * 128], ident[:])
                                xt = bpool.tile([128, 2 * DIN], BF16, tag="xts", bufs=8)
                                nc.scalar.copy(xt[:].bitcast(F32), ps_xt[:].bitcast(F32))
                                ps_xw = pspool.tile([128, 2 * DOUT], F32, tag="xw")
                                for half in range(2):
                                    for k in range(4):
                                        nc.tensor.matmul(
                                            ps_xw[:, half * DOUT:(half + 1) * DOUT],
                                            xt[:, half * DIN + k * 128:
                                               half * DIN + (k + 1) * 128],
                                            wg[:, k * DOUT:(k + 1) * DOUT],
                                            start=(k == 0), stop=(k == 3))
                                xw = bpool.tile([128, 2 * DOUT], BF16, tag="xws", bufs=12)
                                nc.scalar.copy(xw[:], ps_xw[:])
                                ps_agg = pspool.tile([128, 2 * DOUT], F32, tag="agg")
                                for half in range(2):
                                    B = B0 + half
                                    nc.tensor.matmul(
                                        ps_agg[:, half * DOUT:(half + 1) * DOUT],
                                        bdt[br][:, B * 128:(B + 1) * 128],
                                        xw[:, half * DOUT:(half + 1) * DOUT],
                                        start=True, stop=True)
                                t0 = bpool.tile([128, 2 * DOUT], BF16, tag="t0")
                                if use_bias:
                                    nc.vector.tensor_tensor(
                                        t0[:].rearrange("p (v c) -> p v c", v=2),
                                        ps_agg[:].rearrange("p (v c) -> p v c", v=2),
                                        bg_bc[:].unsqueeze(1).broadcast_to(
                                            (128, 2, DOUT)), AX.add)
                                else:
                                    nc.scalar.copy(t0[:], ps_agg[:])
                                t2 = bpool.tile([128, 2 * DOUT], BF16, tag="t2")
                                nc.vector.tensor_scalar_mul(t2[:], t0[:], a_rep[:, 0:1])
                                h = bpool.tile([128, 2 * DOUT], BF16, tag="h")
                                nc.vector.tensor_tensor(h[:], t0[:], t2[:], AX.max)
                                if ps_pt is None:
                                    ps_pt = [pqpool.tile([128, 512], F32,
                                                         name=f"pt{dc}", tag=f"pt{dc}")
                                             for dc in range(2)]
                                for half in range(2):
                                    bi = (B0 + half) % 16
                                    for dc in range(2):
                                        nc.tensor.matmul(
                                            ps_pt[dc][:, bi * 32:(bi + 1) * 32],
                                            h[:, half * DOUT + dc * 128:
                                              half * DOUT + (dc + 1) * 128], pam[:],
                                            start=True, stop=True)
                            if t % 2 == 1:
                                g = t // 2
                                for dc in range(2):
                                    nc.scalar.copy(
                                        poolt[br][dc][:, g * 512:(g + 1) * 512],
                                        ps_pt[dc][:])
                                ps_pt = None
            # =============== bilinear + norms + scores ===============
            # poolt cols: 512*g + 32*m + j (pool) / +16 (anchor); b = 256*g+16*m+j
            def quarter(br, dc, bg, anchor):
                # strided AP covering b in [512*bg, 512*bg+512), linear in (gg,m,j)
                full = poolt[br][dc][:].rearrange(
                    "p (g m t) -> p g m t", g=4, m=16, t=32)
                tsl = slice(16, 32) if anchor else slice(0, 16)
                return full[:, 2 * bg:2 * bg + 2, :, tsl]

            with tc.tile_pool(name="bil", bufs=2) as lpool, \
                 tc.tile_pool(name="psb", bufs=2, space="PSUM") as psb, \
                 tc.tile_pool(name="pss", bufs=1, space="PSUM") as pss:
                for bg in range(2):
                    # uT = W_bil.T-chunks.T @ anchorT  -> linear-b cols
                    ut_sb = []
                    for dc in range(2):
                        ps_ut = psb.tile([128, 512], F32, tag="ut")
                        for ec in range(2):
                            nc.tensor.matmul(
                                ps_ut[:], wbt[:, ec * 256 + dc * 128:
                                              ec * 256 + (dc + 1) * 128],
                                quarter("pos", ec, bg, True),
                                start=(ec == 0), stop=(ec == 1))
                        u = lpool.tile([128, 512], BF16, tag=f"ut{dc}")
                        nc.scalar.copy(u[:], ps_ut[:])
                        ut_sb.append(u)

                    def lin3(ap):
                        return ap.rearrange("p (gg m j) -> p gg m j", gg=2, m=16)

                    names = ("ssa", "ssp", "ssn", "rwp", "rwn")
                    ps_v = {n: pss.tile([1, 512], F32, name=n, tag=n)
                            for n in names}
                    for dc in range(2):
                        sqa = lpool.tile([128, 512], BF16, tag="sqa")
                        qa = quarter("pos", dc, bg, True)
                        nc.vector.tensor_tensor(lin3(sqa[:]), qa, qa, AX.mult)
                        sqp = lpool.tile([128, 512], BF16, tag="sqp")
                        qp = quarter("pos", dc, bg, False)
                        nc.vector.tensor_tensor(lin3(sqp[:]), qp, qp, AX.mult)
                        sqn = lpool.tile([128, 512], BF16, tag="sqn")
                        qn = quarter("neg", dc, bg, False)
                        nc.vector.tensor_tensor(lin3(sqn[:]), qn, qn, AX.mult)
                        prp = lpool.tile([128, 512], BF16, tag="prp")
                        nc.vector.tensor_tensor(
                            lin3(prp[:]), qp, lin3(ut_sb[dc][:]), AX.mult)
                        prn = lpool.tile([128, 512], BF16, tag="prn")
                        nc.vector.tensor_tensor(
                            lin3(prn[:]), qn, lin3(ut_sb[dc][:]), AX.mult)
                        for n, sq in (("ssa", sqa), ("ssp", sqp), ("ssn", sqn),
                                      ("rwp", prp), ("rwn", prn)):
                            nc.tensor.matmul(ps_v[n][:], ones_col[:], sq[:],
                                             start=(dc == 0), stop=(dc == 1))
                    # relayout [1,512] -> [128,4] and finish scalar math
                    vec = {}
                    for n in names:
                        row = lpool.tile([1, 512], F32, tag=f"row_{n}")
                        nc.scalar.copy(row[:], ps_v[n][:])
                        v = lpool.tile([128, 4], F32, tag=f"v_{n}")
                        nc.sync.dma_start(v[:], row[:])
                        vec[n] = v
                    na = lpool.tile([128, 4], F32, tag="na")
                    nc.scalar.sqrt(na[:], vec["ssa"][:])
                    nc.vector.tensor_scalar_max(na[:], na[:], EPS)
                    for n, rawn, outrow in (("ssp", "rwp", 0), ("ssn", "rwn", 1)):
                        nn = lpool.tile([128, 4], F32, tag=f"nn{outrow}")
                        nc.scalar.sqrt(nn[:], vec[n][:])
                        nc.vector.tensor_scalar_max(nn[:], nn[:], EPS)
                        nc.vector.tensor_tensor(nn[:], nn[:], na[:], AX.mult)
                        rec = lpool.tile([128, 4], F32, tag=f"rec{outrow}")
                        nc.vector.reciprocal(rec[:], nn[:])
                        sc = lpool.tile([128, 4], F32, tag=f"sc{outrow}")
                        nc.vector.scalar_tensor_tensor(
                            sc[:], vec[rawn][:], 0.0, rec[:],
                            AX.bypass, AX.mult)
                        nc.vector.tensor_scalar_add(sc[:], sc[:], bbil[:, 0:1])
                        nc.sync.dma_start(
                            scores_out[outrow:outrow + 1,
                                       bg * 512:(bg + 1) * 512], sc[:])

    nc.finalize()
    return nc


def _prep(inputs):
    """Host-side marshalling: shard + layout + dtype prep for the 8 cores."""
    bf = ml_dtypes.bfloat16
    pos_x = np.ascontiguousarray(inputs["pos_x"], dtype=np.float32)
    neg_x = np.ascontiguousarray(inputs["neg_x"], dtype=np.float32)

    def edge_prep(src, dst, w):
        c = ((np.asarray(src).astype(np.int64) % S) * S
             + (np.asarray(dst).astype(np.int64) % S)).reshape(B_TOT, EPB)
        wv = np.asarray(w, dtype=np.float32).reshape(B_TOT, EPB)
        return c, wv

    cpos, wpos = edge_prep(inputs["pos_src"], inputs["pos_dst"], inputs["pos_w"])
    cneg, wneg = edge_prep(inputs["neg_src"], inputs["neg_dst"], inputs["neg_w"])

    def tile_layout(arr_k):  # [BC, EPB] -> [128, HT*EPB]
        return np.ascontiguousarray(
            arr_k.reshape(HT, 128, EPB).transpose(1, 0, 2).reshape(128, HT * EPB))

    wg = np.asarray(inputs["W_gcn"], np.float32).astype(bf)
    wg_sb = np.ascontiguousarray(
        wg.reshape(4, 128, DOUT).transpose(1, 0, 2).reshape(128, 4 * DOUT))
    wbt = np.asarray(inputs["W_bil"], np.float32).T.astype(bf)   # [e, d]
    wbt_sb = np.ascontiguousarray(
        wbt.reshape(2, 128, 2, 128).transpose(1, 0, 2, 3).reshape(128, 512))
    pam = np.zeros((128, 32), np.float32)
    for j in range(16):
        pam[S * j:S * j + 7, j] = 1.0 / 7.0
        pam[S * j + 7, 16 + j] = 1.0
    iota = np.tile(np.repeat(np.arange(EPB, dtype=np.float32), EPB)[None, :],
                   (128, 1))
    a_rep = np.full((128, 1), float(np.asarray(inputs["prelu_a"])), np.float32)
    bbil_rep = np.full((128, 1), float(np.asarray(inputs["b_bil"]).ravel()[0]),
                       np.float32)
    bgcn = np.asarray(inputs["b_gcn"], np.float32).reshape(1, DOUT)
    use_bias = bool(np.any(bgcn))

    consts = {
        "wg_in": wg_sb.astype(bf), "wbt_in": wbt_sb.astype(bf),
        "pam_in": pam.astype(bf), "iota_in": iota.astype(bf),
        "a_in": a_rep, "bbil_in": bbil_rep,
    }
    if use_bias:
        consts["bgcn_in"] = bgcn

    in_maps = []
    for k in range(N_CORES):
        bs = slice(k * BC, (k + 1) * BC)
        ns = slice(k * NC_NODES, (k + 1) * NC_NODES)
        m = dict(consts)
        m["x_pos"] = pos_x[ns]
        m["x_neg"] = neg_x[ns]
        m["cidx_pos"] = tile_layout(cpos[bs]).astype(bf)
        m["cidx_neg"] = tile_layout(cneg[bs]).astype(bf)
        m["ew_pos"] = tile_layout(wpos[bs]).astype(bf)
        m["ew_neg"] = tile_layout(wneg[bs]).astype(bf)
        in_maps.append(m)
    return in_maps, use_bias


def kernel(**inputs):
    in_maps, use_bias = _prep(inputs)
    if use_bias not in _KERNEL_CACHE:
        _KERNEL_CACHE[use_bias] = _build(use_bias)
    nc = _KERNEL_CACHE[use_bias]
    res = run_bass_kernel_spmd(nc, in_maps, core_ids=list(range(N_CORES)))
    pos = np.concatenate([r["scores_out"][0] for r in res.results])
    neg = np.concatenate([r["scores_out"][1] for r in res.results])
    return pos, neg



# revision 32
# speedup vs baseline: 2.1621x; 2.1621x over previous
"""CoLA GNN model kernel for 8 Trainium2 NeuronCores.

Math (per branch, pos/neg):
  xw   = x @ W_gcn                                   [N, 256]
  agg  = scatter_add(dst, w * xw[src])               [N, 256]
  h    = PReLU(agg + b_gcn)                          [N, 256]
  pool = l2norm(mean(h over nodes 0..6 per subgraph))
  anch = l2norm(h node 7 per subgraph)               (pos branch only)
  score_b = pool_b . (W_bil @ anch_b) + b_bil

Device mapping (per core: 1024 subgraphs = 8192 nodes per branch):
  - host precomputes x^T (bf16, feature-chunk-major) and the weighted
    block-diagonal adjacency bdt[src, dst] per 128-node block (bf16);
  - per 256-node pair: xw on PE (lhsT = x^T chunk), PSUM->SBUF copy on
    Act, agg via block-diag matmul on PE, PReLU fused on DVE,
    pool/anchor transposed out via h-stationary matmuls;
  - stages software-pipelined with a skew of one/two pairs; poolt
    group copies staggered across two pairs to keep Act under PE;
  - bilinear products/ut hoisted into the pair loop's engine slack;
    device emits the 5 raw reduction sums per subgraph (via indicator
    matmuls into one [5,512] PSUM tile per half-batch); the final
    score = rw / (||pool|| * ||anch||) + b is done on host.
"""

import numpy as np
import ml_dtypes

import concourse.mybir as mybir
import concourse.tile as tile
from concourse import bacc
from concourse.bass_utils import run_bass_kernel_spmd

F32 = mybir.dt.float32
BF16 = mybir.dt.bfloat16
AX = mybir.AluOpType

N_CORES = 8
S = 8                     # nodes per subgraph
B_TOT = 8192              # subgraphs total
BC = B_TOT // N_CORES     # subgraphs per core (1024)
NC_NODES = BC * S         # nodes per core (8192)
DIN = 512
DOUT = 256
EPB = 64                  # edges per subgraph
NBLK = NC_NODES // 128    # 64 row-blocks of 128 nodes (16 subgraphs) per branch
NPAIR = NBLK // 2         # 32 block-pairs per branch
NSLAB = 16                # x^T slabs per branch (512 nodes each)
EPS = 1e-12
SUMS = ("ssa", "ssp", "ssn", "rwp", "rwn")

_KERNEL_CACHE = {}


def _build(use_bias: bool):
    nc = bacc.Bacc(None, target_bir_lowering=False)

    # ---- I/O ----
    xt_pos = nc.dram_tensor("xt_pos", [128, 4 * NC_NODES], BF16, kind="ExternalInput")
    xt_neg = nc.dram_tensor("xt_neg", [128, 4 * NC_NODES], BF16, kind="ExternalInput")
    bdt_pos = nc.dram_tensor("bdt_pos", [128, NBLK * 128], BF16, kind="ExternalInput")
    bdt_neg = nc.dram_tensor("bdt_neg", [128, NBLK * 128], BF16, kind="ExternalInput")
    wg_in = nc.dram_tensor("wg_in", [128, 4 * DOUT], BF16, kind="ExternalInput")
    wbt_in = nc.dram_tensor("wbt_in", [128, 512], BF16, kind="ExternalInput")
    pam_in = nc.dram_tensor("pam_in", [128, 32], BF16, kind="ExternalInput")
    ind5_in = nc.dram_tensor("ind5_in", [128, 25], BF16, kind="ExternalInput")
    a_in = nc.dram_tensor("a_in", [128, 1], F32, kind="ExternalInput")
    if use_bias:
        bgcn_in = nc.dram_tensor("bgcn_in", [1, DOUT], F32, kind="ExternalInput")
    sums_out = nc.dram_tensor("sums_out", [2, 5 * 512], F32, kind="ExternalOutput")

    with tile.TileContext(nc) as tc:
        with tc.tile_pool(name="const", bufs=1) as cpool, \
             tc.tile_pool(name="persist", bufs=1) as ppool:

            wg = cpool.tile([128, 4 * DOUT], BF16)
            wbt = cpool.tile([128, 512], BF16)
            pam = cpool.tile([128, 32], BF16)
            ind5 = cpool.tile([128, 25], BF16)
            a_rep = cpool.tile([128, 1], F32)
            if use_bias:
                bg_row = cpool.tile([1, DOUT], F32)
                bg_bc = cpool.tile([128, DOUT], F32)

            bdt = {}
            poolt = {}
            for br in ("pos", "neg"):
                bdt[br] = ppool.tile([128, NBLK * 128], BF16,
                                     name=f"bdt_{br}", tag=f"bdt_{br}")
                poolt[br] = [
                    ppool.tile([128, (NBLK // 16) * 512], BF16,
                               name=f"poolt_{br}{dc}", tag=f"poolt_{br}{dc}")
                    for dc in range(2)
                ]
            # bilinear intermediates (persistent, written mid-loop)
            ut_sb = {(bg, dc): ppool.tile([128, 512], BF16,
                                          name=f"ut{bg}{dc}", tag=f"ut{bg}{dc}")
                     for bg in range(2) for dc in range(2)}
            rs = {(n, bg): ppool.tile([128, 512], BF16,
                                      name=f"rs_{n}{bg}", tag=f"rs_{n}{bg}")
                  for n in SUMS for bg in range(2)}
            btmp = [ppool.tile([128, 512], BF16, name=f"btmp{i}", tag=f"btmp{i}")
                    for i in range(2)]

            branches = ("pos", "neg")
            xt_dram = {"pos": xt_pos, "neg": xt_neg}
            bdt_dram = {"pos": bdt_pos, "neg": bdt_neg}

            def dma_bdt(br, blo, bhi):
                # gpsimd (SWDGE) queue: keeps bdt transfers out of the
                # SP FIFO so x^T slab prefetches aren't queued behind them
                nc.gpsimd.dma_start(bdt[br][:, blo * 128:bhi * 128],
                                    bdt_dram[br][:, blo * 128:bhi * 128])

            # poolt cols: 512*g + 32*m + j (pool) / +16 (anchor); b = 256*g+16*m+j
            def quarter(br, dc, bg, anchor):
                full = poolt[br][dc][:].rearrange(
                    "p (g m t) -> p g m t", g=4, m=16, t=32)
                tsl = slice(16, 32) if anchor else slice(0, 16)
                return full[:, 2 * bg:2 * bg + 2, :, tsl]

            def lin3(ap):
                return ap.rearrange("p (gg m j) -> p gg m j", gg=2, m=16)

            with tc.tile_pool(name="xp", bufs=3) as xpool, \
                 tc.tile_pool(name="blk", bufs=4) as bpool, \
                 tc.tile_pool(name="ps", bufs=2, space="PSUM") as pspool, \
                 tc.tile_pool(name="psq", bufs=2, space="PSUM") as pqpool:

                xt_tiles = {}

                def _slab(br, s):
                    t = xpool.tile([128, 4 * 512], BF16,
                                   name=f"xt_{br}{s}", tag="xt", bufs=16)
                    src = xt_dram[br][:].rearrange("p (c n) -> p c n", c=4)
                    nc.sync.dma_start(
                        t[:].rearrange("p (c n) -> p c n", c=4),
                        src[:, :, s * 512:(s + 1) * 512])
                    xt_tiles[(br, s)] = t

                # ---- bilinear hoisted pieces ----
                def bil_ut(bg, dc):
                    ps_ut = pspool.tile([128, 512], F32,
                                        name=f"ps_ut{bg}{dc}", tag="xw")
                    for ec in range(2):
                        nc.tensor.matmul(
                            ps_ut[:], wbt[:, ec * 256 + dc * 128:
                                          ec * 256 + (dc + 1) * 128],
                            quarter("pos", ec, bg, True),
                            start=(ec == 0), stop=(ec == 1))
                    nc.scalar.copy(ut_sb[(bg, dc)][:], ps_ut[:])

                def bil_prod(n, bg):
                    # rs[n,bg] = sum over dc of elementwise product
                    qa = {"ssa": lambda dc: quarter("pos", dc, bg, True),
                          "ssp": lambda dc: quarter("pos", dc, bg, False),
                          "ssn": lambda dc: quarter("neg", dc, bg, False),
                          "rwp": lambda dc: quarter("pos", dc, bg, False),
                          "rwn": lambda dc: quarter("neg", dc, bg, False)}[n]
                    for dc in range(2):
                        q = qa(dc)
                        other = (lin3(ut_sb[(bg, dc)][:])
                                 if n in ("rwp", "rwn") else q)
                        nc.vector.tensor_tensor(
                            lin3(btmp[dc][:]), q, other, AX.mult)
                    nc.vector.tensor_tensor(
                        rs[(n, bg)][:], btmp[0][:], btmp[1][:], AX.add)

                def bil_prod_g(n, bg, gg):
                    # one 256-col group-half of bil_prod (gg in {0,1} within
                    # the half-batch); lets the last half run post-loop only
                    br = "neg" if n in ("ssn", "rwn") else "pos"
                    anchor = n == "ssa"
                    cs = slice(gg * 256, (gg + 1) * 256)
                    tsl = slice(16, 32) if anchor else slice(0, 16)
                    for dc in range(2):
                        full = poolt[br][dc][:].rearrange(
                            "p (g m t) -> p g m t", g=4, m=16, t=32)
                        q = full[:, 2 * bg + gg:2 * bg + gg + 1, :, tsl]
                        if n in ("rwp", "rwn"):
                            other = lin3(ut_sb[(bg, dc)][:])[:, gg:gg + 1]
                        else:
                            other = q
                        nc.vector.tensor_tensor(
                            lin3(btmp[dc][:])[:, gg:gg + 1], q, other, AX.mult)
                    nc.vector.tensor_tensor(
                        rs[(n, bg)][:, cs], btmp[0][:, cs], btmp[1][:, cs],
                        AX.add)

                ps_sums = {}

                def bil_sums(bg, part="all"):
                    # 5 reductions into one [5,512] PSUM via indicator lhsT;
                    # "pos"/"neg" split lets the pos-side matmuls fire
                    # before the neg products land.
                    if part in ("all", "pos"):
                        ps_sums[bg] = pqpool.tile([128, 512], F32,
                                                  name=f"ps_sums{bg}",
                                                  tag="pt0")
                    ps_s = ps_sums[bg]
                    idx = {"all": (0, 1, 2, 3, 4), "pos": (0, 1, 3),
                           "neg": (2, 4)}[part]
                    for k in idx:
                        nc.tensor.matmul(ps_s[0:5, :],
                                         ind5[:, 5 * k:5 * k + 5],
                                         rs[(SUMS[k], bg)][:],
                                         start=(k == idx[0] and
                                                part in ("all", "pos")),
                                         stop=(k == idx[-1] and
                                               part in ("all", "neg")))
                    if part in ("all", "neg"):
                        ssb = bpool.tile([5, 512], F32, name=f"sums_sb{bg}",
                                         tag="sums_sb", bufs=2)
                        nc.scalar.copy(ssb[:], ps_s[0:5, :])
                        nc.sync.dma_start(
                            sums_out[bg:bg + 1, :].rearrange(
                                "r (p c) -> r p c", p=5),
                            ssb[:])

                # schedule[g] = thunks emitted just before pair g
                NG = 2 * NPAIR
                schedule = [[] for _ in range(NG + 4)]
                schedule[0].append(
                    lambda: nc.gpsimd.dma_start(pam[:], pam_in[:]))
                schedule[0].append(
                    lambda: nc.gpsimd.dma_start(wbt[:], wbt_in[:]))
                schedule[0].append(
                    lambda: nc.gpsimd.dma_start(ind5[:], ind5_in[:]))
                # bdt in 8-block chunks, spread to smooth DMA-bus load
                for c in range(1, 8):
                    schedule[max(0, 4 * c - 12)].append(
                        lambda lo=8 * c: dma_bdt("pos", lo, lo + 8))
                for c in range(8):
                    schedule[4 * c + 18].append(
                        lambda lo=8 * c: dma_bdt("neg", lo, lo + 8))
                for b in range(2):
                    for s in range(NSLAB):
                        g = 32 * b + 2 * s - 16
                        if g >= 0:
                            schedule[g].append(
                                lambda br=branches[b], s=s: _slab(br, s))
                # hoisted bilinear work (deps: pos poolt grp0/1 by g~18,
                # grp2/3 by g~36; neg grp0/1 by g~50). Positions avoid the
                # post-group-boundary pairs where Act does poolt copies.
                schedule[20].append(lambda: bil_ut(0, 0))
                schedule[21].append(lambda: bil_ut(0, 1))
                schedule[22].append(lambda: bil_prod("ssa", 0))
                schedule[23].append(lambda: bil_prod("ssp", 0))
                schedule[24].append(lambda: bil_prod("rwp", 0))
                schedule[36].append(lambda: bil_ut(1, 0))
                schedule[37].append(lambda: bil_ut(1, 1))
                schedule[38].append(lambda: bil_prod("ssa", 1))
                schedule[39].append(lambda: bil_prod("ssp", 1))
                schedule[40].append(lambda: bil_prod("rwp", 1))
                schedule[52].append(lambda: bil_prod("ssn", 0))
                schedule[53].append(lambda: bil_prod("rwn", 0))
                schedule[55].append(lambda: bil_sums(0))
                schedule[61].append(lambda: bil_prod_g("ssn", 1, 0))
                schedule[62].append(lambda: bil_prod_g("rwn", 1, 0))

                # head: minimal serial prefix + PE p-state warmup. ~400
                # tiny matmuls keep PE continuously busy through the DMA
                # head so real matmuls start fully ramped.
                warm = bpool.tile([128, 16], BF16, name="warm", tag="warm")
                nc.vector.memset(warm[:], 0.0)
                one_f = bpool.tile([128, 1], F32, name="one_f", tag="one_f")
                nc.vector.memset(one_f[:], 1.0)
                ps_warm = pqpool.tile([128, 512], F32, name="ps_warm",
                                      tag="pt0")
                for _ in range(400):
                    nc.tensor.matmul(ps_warm[0:16, 0:16], warm[:],
                                     warm[:], start=True, stop=True)
                # dummy reader so the verifier sees ps_warm consumed
                nc.vector.tensor_copy(btmp[0][0:16, 0:16],
                                      ps_warm[0:16, 0:16])
                nc.sync.dma_start(wg[:], wg_in[:])
                _slab("pos", 0)
                _slab("pos", 1)
                dma_bdt("pos", 0, 8)
                nc.sync.dma_start(a_rep[:], a_in[:])
                if use_bias:
                    nc.sync.dma_start(bg_row[:], bgcn_in[:])
                    nc.gpsimd.partition_broadcast(bg_bc[:], bg_row[:])
                for s in range(2, 8):
                    _slab("pos", s)

                state = {}
                ps_pt = {}
                pending_copy = []   # staggered poolt copies

                def stage_xw(g):
                    br = branches[g // NPAIR]
                    B0 = 2 * (g % NPAIR)
                    xt = xt_tiles[(br, B0 // 4)]
                    ps_xw = pspool.tile([128, 2 * DOUT], F32,
                                        name=f"ps_xw{g}", tag="xw")
                    for half in range(2):
                        bb = (B0 + half) % 4
                        for k in range(4):
                            nc.tensor.matmul(
                                ps_xw[:, half * DOUT:(half + 1) * DOUT],
                                xt[:, k * 512 + bb * 128:
                                   k * 512 + (bb + 1) * 128],
                                wg[:, k * DOUT:(k + 1) * DOUT],
                                start=(k == 0), stop=(k == 3))
                    xw_sb = bpool.tile([128, 2 * DOUT], BF16,
                                       name=f"xw_sb{g}", tag="xw_sb", bufs=4)
                    nc.vector.tensor_copy(xw_sb[:], ps_xw[:])
                    state[g] = {"br": br, "B0": B0, "xw_sb": xw_sb}

                def stage_agg(g):
                    st = state[g]
                    br, B0, xw_sb = st["br"], st["B0"], st["xw_sb"]
                    ps_agg = pspool.tile([128, 2 * DOUT], F32,
                                         name=f"ps_agg{g}", tag="agg")
                    for half in range(2):
                        B = B0 + half
                        nc.tensor.matmul(
                            ps_agg[:, half * DOUT:(half + 1) * DOUT],
                            bdt[br][:, B * 128:(B + 1) * 128],
                            xw_sb[:, half * DOUT:(half + 1) * DOUT],
                            start=True, stop=True)
                    h = bpool.tile([128, 2 * DOUT], BF16,
                                   name=f"h{g}", tag="h", bufs=4)
                    if use_bias:
                        t0 = bpool.tile([128, 2 * DOUT], BF16,
                                        name=f"t0_{g}", tag="t0", bufs=4)
                        nc.vector.tensor_tensor(
                            t0[:].rearrange("p (v c) -> p v c", v=2),
                            ps_agg[:].rearrange("p (v c) -> p v c", v=2),
                            bg_bc[:].unsqueeze(1).broadcast_to((128, 2, DOUT)),
                            AX.add)
                        nc.scalar.activation(
                            h[:], t0[:], mybir.ActivationFunctionType.Prelu,
                            alpha=a_rep[:, 0:1])
                    else:
                        nc.scalar.activation(
                            h[:], ps_agg[:],
                            mybir.ActivationFunctionType.Prelu,
                            alpha=a_rep[:, 0:1])
                    st["h"] = h

                def flush_pending():
                    # last group's copies: split across Act/DVE to shorten
                    # the tail dependency chain
                    while pending_copy:
                        br_, grp_, dc_, pt_ = pending_copy.pop(0)
                        dst = poolt[br_][dc_][:, grp_ * 512:(grp_ + 1) * 512]
                        if dc_ == 0:
                            nc.scalar.copy(dst, pt_[dc_][:])
                        else:
                            nc.vector.tensor_copy(dst, pt_[dc_][:])
                            del ps_pt[(br_, grp_)]

                def stage_pool(g):
                    if pending_copy:
                        br_, grp_, dc_, pt_ = pending_copy.pop(0)
                        nc.scalar.copy(
                            poolt[br_][dc_][:, grp_ * 512:(grp_ + 1) * 512],
                            pt_[dc_][:])
                        if dc_ == 1:
                            del ps_pt[(br_, grp_)]
                    st = state.pop(g)
                    br, B0, h = st["br"], st["B0"], st["h"]
                    grp = B0 // 16
                    if (br, grp) not in ps_pt:
                        ps_pt[(br, grp)] = [
                            pqpool.tile([128, 512], F32,
                                        name=f"pt{dc}_{br}{grp}", tag=f"pt{dc}")
                            for dc in range(2)]
                    pt = ps_pt[(br, grp)]
                    for half in range(2):
                        bi = (B0 + half) % 16
                        for dc in range(2):
                            nc.tensor.matmul(
                                pt[dc][:, bi * 32:(bi + 1) * 32],
                                h[:, half * DOUT + dc * 128:
                                  half * DOUT + (dc + 1) * 128],
                                pam[:], start=True, stop=True)
                    if B0 % 16 == 14:
                        pending_copy.append((br, grp, 0, pt))
                        pending_copy.append((br, grp, 1, pt))

                for g in range(NG + 2):
                    if g < len(schedule):
                        for th in schedule[g]:
                            th()
                    if g < NG:
                        stage_xw(g)
                    if 1 <= g <= NG:
                        stage_agg(g - 1)
                    if g >= 2:
                        stage_pool(g - 2)
                flush_pending()
                # ---- tail: bg1 last-group products + reductions + out ----
                bil_sums(1, "pos")
                bil_prod_g("ssn", 1, 1)
                bil_prod_g("rwn", 1, 1)
                bil_sums(1, "neg")

    nc.finalize()
    return nc


def _prep(inputs):
    """Host-side marshalling: shard + layout + dtype prep for the 8 cores."""
    bf = ml_dtypes.bfloat16

    def xt_prep(x):
        xb = np.asarray(x, np.float32).astype(bf).view(np.uint16)
        xb = xb.reshape(N_CORES, NC_NODES, 4, 128).transpose(0, 3, 2, 1)
        return np.ascontiguousarray(xb).reshape(N_CORES, 128, 4 * NC_NODES) \
            .view(bf)

    def bdt_prep(src, dst, w):
        src = np.asarray(src).astype(np.int64)
        dst = np.asarray(dst).astype(np.int64)
        w = np.asarray(w, np.float64)
        sub = src // S
        c = (src % S) * S + (dst % S)
        A = np.bincount(sub * EPB + c, weights=w,
                        minlength=B_TOT * EPB).astype(np.float32)
        A8 = A.reshape(N_CORES, NBLK, 16, S, S)      # [core, B, j, s, d]
        out = np.zeros((N_CORES, NBLK, 16, S, 16, S), np.float32)
        for j in range(16):
            out[:, :, j, :, j, :] = A8[:, :, j]
        out = out.transpose(0, 2, 3, 1, 4, 5).reshape(N_CORES, 128, NBLK * 128)
        return np.ascontiguousarray(out).astype(bf)

    xt_pos = xt_prep(inputs["pos_x"])
    xt_neg = xt_prep(inputs["neg_x"])
    bdt_pos = bdt_prep(inputs["pos_src"], inputs["pos_dst"], inputs["pos_w"])
    bdt_neg = bdt_prep(inputs["neg_src"], inputs["neg_dst"], inputs["neg_w"])

    wg = np.asarray(inputs["W_gcn"], np.float32).astype(bf)
    wg_sb = np.ascontiguousarray(
        wg.reshape(4, 128, DOUT).transpose(1, 0, 2).reshape(128, 4 * DOUT))
    wbt = np.asarray(inputs["W_bil"], np.float32).T.astype(bf)   # [e, d]
    wbt_sb = np.ascontiguousarray(
        wbt.reshape(2, 128, 2, 128).transpose(1, 0, 2, 3).reshape(128, 512))
    pam = np.zeros((128, 32), np.float32)
    for j in range(16):
        pam[S * j:S * j + 7, j] = 1.0 / 7.0
        pam[S * j + 7, 16 + j] = 1.0
    ind5 = np.zeros((5, 5), np.float32)
    np.fill_diagonal(ind5, 1.0)
    ind5 = np.tile(ind5.reshape(1, 25), (128, 1))
    a_rep = np.full((128, 1), float(np.asarray(inputs["prelu_a"])), np.float32)
    bgcn = np.asarray(inputs["b_gcn"], np.float32).reshape(1, DOUT)
    use_bias = bool(np.any(bgcn))

    consts = {
        "wg_in": wg_sb.astype(bf), "wbt_in": wbt_sb.astype(bf),
        "pam_in": pam.astype(bf), "ind5_in": ind5.astype(bf),
        "a_in": a_rep,
    }
    if use_bias:
        consts["bgcn_in"] = bgcn

    in_maps = []
    for k in range(N_CORES):
        m = dict(consts)
        m["xt_pos"] = xt_pos[k]
        m["xt_neg"] = xt_neg[k]
        m["bdt_pos"] = bdt_pos[k]
        m["bdt_neg"] = bdt_neg[k]
        in_maps.append(m)
    return in_maps, use_bias


def kernel(**inputs):
    in_maps, use_bias = _prep(inputs)
    if use_bias not in _KERNEL_CACHE:
        _KERNEL_CACHE[use_bias] = _build(use_bias)
    nc = _KERNEL_CACHE[use_bias]
    res = run_bass_kernel_spmd(nc, in_maps, core_ids=list(range(N_CORES)))
    bbil = float(np.asarray(inputs["b_bil"]).ravel()[0])
    pos_parts, neg_parts = [], []
    for r in res.results:
        s = np.asarray(r["sums_out"], np.float64).reshape(2, 5, 512)
        ssa, ssp, ssn, rwp, rwn = (s[:, i, :] for i in range(5))  # [2, 512]
        na = np.maximum(np.sqrt(ssa), EPS)
        pos = rwp / (np.maximum(np.sqrt(ssp), EPS) * na) + bbil
        neg = rwn / (np.maximum(np.sqrt(ssn), EPS) * na) + bbil
        pos_parts.append(pos.reshape(-1))
        neg_parts.append(neg.reshape(-1))
    pos = np.concatenate(pos_parts).astype(np.float32)
    neg = np.concatenate(neg_parts).astype(np.float32)
    return pos, neg


# revision 34
# speedup vs baseline: 2.1981x; 1.0167x over previous
"""CoLA GNN model kernel for 8 Trainium2 NeuronCores.

Math (per branch, pos/neg):
  xw   = x @ W_gcn                                   [N, 256]
  agg  = scatter_add(dst, w * xw[src])               [N, 256]
  h    = PReLU(agg + b_gcn)                          [N, 256]
  pool = l2norm(mean(h over nodes 0..6 per subgraph))
  anch = l2norm(h node 7 per subgraph)               (pos branch only)
  score_b = pool_b . (W_bil @ anch_b) + b_bil

Device mapping (per core: 1024 subgraphs = 8192 nodes per branch):
  - host precomputes x^T (bf16, feature-chunk-major) and the weighted
    block-diagonal adjacency bdt[src, dst] per 128-node block (bf16);
  - per 256-node pair: xw on PE (lhsT = x^T chunk), PSUM->SBUF copy on
    Act, agg via block-diag matmul on PE, PReLU fused on DVE,
    pool/anchor transposed out via h-stationary matmuls;
  - stages software-pipelined with a skew of one/two pairs; poolt
    group copies staggered across two pairs to keep Act under PE;
  - bilinear products/ut hoisted into the pair loop's engine slack;
    device emits the 5 raw reduction sums per subgraph (via indicator
    matmuls into one [5,512] PSUM tile per half-batch); the final
    score = rw / (||pool|| * ||anch||) + b is done on host.
"""

import numpy as np
import ml_dtypes

import concourse.mybir as mybir
import concourse.tile as tile
from concourse import bacc
from concourse.bass_utils import run_bass_kernel_spmd

F32 = mybir.dt.float32
BF16 = mybir.dt.bfloat16
AX = mybir.AluOpType

N_CORES = 8
S = 8                     # nodes per subgraph
B_TOT = 8192              # subgraphs total
BC = B_TOT // N_CORES     # subgraphs per core (1024)
NC_NODES = BC * S         # nodes per core (8192)
DIN = 512
DOUT = 256
EPB = 64                  # edges per subgraph
NBLK = NC_NODES // 128    # 64 row-blocks of 128 nodes (16 subgraphs) per branch
NPAIR = NBLK // 2         # 32 block-pairs per branch
NSLAB = 16                # x^T slabs per branch (512 nodes each)
EPS = 1e-12
SUMS = ("ssa", "ssp", "ssn", "rwp", "rwn")

_KERNEL_CACHE = {}


def _build(use_bias: bool):
    nc = bacc.Bacc(None, target_bir_lowering=False)

    # ---- I/O ----
    xt_pos = nc.dram_tensor("xt_pos", [128, 4 * NC_NODES], BF16, kind="ExternalInput")
    xt_neg = nc.dram_tensor("xt_neg", [128, 4 * NC_NODES], BF16, kind="ExternalInput")
    bdt_pos = nc.dram_tensor("bdt_pos", [128, NBLK * 128], BF16, kind="ExternalInput")
    bdt_neg = nc.dram_tensor("bdt_neg", [128, NBLK * 128], BF16, kind="ExternalInput")
    wg_in = nc.dram_tensor("wg_in", [128, 4 * DOUT], BF16, kind="ExternalInput")
    wbt_in = nc.dram_tensor("wbt_in", [128, 512], BF16, kind="ExternalInput")
    pam_in = nc.dram_tensor("pam_in", [128, 32], BF16, kind="ExternalInput")
    ind5_in = nc.dram_tensor("ind5_in", [128, 25], BF16, kind="ExternalInput")
    a_in = nc.dram_tensor("a_in", [128, 1], F32, kind="ExternalInput")
    if use_bias:
        bgcn_in = nc.dram_tensor("bgcn_in", [1, DOUT], F32, kind="ExternalInput")
    sums_out = nc.dram_tensor("sums_out", [2, 5 * 512], F32, kind="ExternalOutput")

    with tile.TileContext(nc) as tc:
        with tc.tile_pool(name="const", bufs=1) as cpool, \
             tc.tile_pool(name="persist", bufs=1) as ppool:

            wg = cpool.tile([128, 4 * DOUT], BF16)
            wbt = cpool.tile([128, 512], BF16)
            pam = cpool.tile([128, 32], BF16)
            ind5 = cpool.tile([128, 25], BF16)
            a_rep = cpool.tile([128, 1], F32)
            if use_bias:
                bg_row = cpool.tile([1, DOUT], F32)
                bg_bc = cpool.tile([128, DOUT], F32)

            bdt = {}
            poolt = {}
            for br in ("pos", "neg"):
                bdt[br] = ppool.tile([128, NBLK * 128], BF16,
                                     name=f"bdt_{br}", tag=f"bdt_{br}")
                poolt[br] = [
                    ppool.tile([128, (NBLK // 16) * 512], BF16,
                               name=f"poolt_{br}{dc}", tag=f"poolt_{br}{dc}")
                    for dc in range(2)
                ]
            # bilinear intermediates (persistent, written mid-loop)
            ut_sb = {(bg, dc): ppool.tile([128, 512], BF16,
                                          name=f"ut{bg}{dc}", tag=f"ut{bg}{dc}")
                     for bg in range(2) for dc in range(2)}
            rs = {(n, bg): ppool.tile([128, 512], BF16,
                                      name=f"rs_{n}{bg}", tag=f"rs_{n}{bg}")
                  for n in SUMS for bg in range(2)}
            btmp = [ppool.tile([128, 512], BF16, name=f"btmp{i}", tag=f"btmp{i}")
                    for i in range(2)]

            branches = ("pos", "neg")
            xt_dram = {"pos": xt_pos, "neg": xt_neg}
            bdt_dram = {"pos": bdt_pos, "neg": bdt_neg}

            def dma_bdt(br, blo, bhi):
                # gpsimd (SWDGE) queue: keeps bdt transfers out of the
                # SP FIFO so x^T slab prefetches aren't queued behind them
                nc.gpsimd.dma_start(bdt[br][:, blo * 128:bhi * 128],
                                    bdt_dram[br][:, blo * 128:bhi * 128])

            # poolt cols: 512*g + 32*m + j (pool) / +16 (anchor); b = 256*g+16*m+j
            def quarter(br, dc, bg, anchor):
                full = poolt[br][dc][:].rearrange(
                    "p (g m t) -> p g m t", g=4, m=16, t=32)
                tsl = slice(16, 32) if anchor else slice(0, 16)
                return full[:, 2 * bg:2 * bg + 2, :, tsl]

            def lin3(ap):
                return ap.rearrange("p (gg m j) -> p gg m j", gg=2, m=16)

            with tc.tile_pool(name="xp", bufs=3) as xpool, \
                 tc.tile_pool(name="blk", bufs=4) as bpool, \
                 tc.tile_pool(name="ps", bufs=2, space="PSUM") as pspool, \
                 tc.tile_pool(name="psq", bufs=2, space="PSUM") as pqpool:

                xt_tiles = {}

                def _slab(br, s):
                    t = xpool.tile([128, 4 * 512], BF16,
                                   name=f"xt_{br}{s}", tag="xt", bufs=16)
                    src = xt_dram[br][:].rearrange("p (c n) -> p c n", c=4)
                    nc.sync.dma_start(
                        t[:].rearrange("p (c n) -> p c n", c=4),
                        src[:, :, s * 512:(s + 1) * 512])
                    xt_tiles[(br, s)] = t

                # ---- bilinear hoisted pieces ----
                def bil_ut(bg, dc):
                    ps_ut = pspool.tile([128, 512], F32,
                                        name=f"ps_ut{bg}{dc}", tag="xw")
                    for ec in range(2):
                        nc.tensor.matmul(
                            ps_ut[:], wbt[:, ec * 256 + dc * 128:
                                          ec * 256 + (dc + 1) * 128],
                            quarter("pos", ec, bg, True),
                            start=(ec == 0), stop=(ec == 1))
                    nc.scalar.copy(ut_sb[(bg, dc)][:], ps_ut[:])

                def bil_prod(n, bg):
                    # rs[n,bg] = sum over dc of elementwise product
                    qa = {"ssa": lambda dc: quarter("pos", dc, bg, True),
                          "ssp": lambda dc: quarter("pos", dc, bg, False),
                          "ssn": lambda dc: quarter("neg", dc, bg, False),
                          "rwp": lambda dc: quarter("pos", dc, bg, False),
                          "rwn": lambda dc: quarter("neg", dc, bg, False)}[n]
                    for dc in range(2):
                        q = qa(dc)
                        other = (lin3(ut_sb[(bg, dc)][:])
                                 if n in ("rwp", "rwn") else q)
                        nc.vector.tensor_tensor(
                            lin3(btmp[dc][:]), q, other, AX.mult)
                    nc.vector.tensor_tensor(
                        rs[(n, bg)][:], btmp[0][:], btmp[1][:], AX.add)

                def bil_prod_g(n, bg, gg):
                    # one 256-col group-half of bil_prod (gg in {0,1} within
                    # the half-batch); lets the last half run post-loop only
                    br = "neg" if n in ("ssn", "rwn") else "pos"
                    anchor = n == "ssa"
                    cs = slice(gg * 256, (gg + 1) * 256)
                    tsl = slice(16, 32) if anchor else slice(0, 16)
                    for dc in range(2):
                        full = poolt[br][dc][:].rearrange(
                            "p (g m t) -> p g m t", g=4, m=16, t=32)
                        q = full[:, 2 * bg + gg:2 * bg + gg + 1, :, tsl]
                        if n in ("rwp", "rwn"):
                            other = lin3(ut_sb[(bg, dc)][:])[:, gg:gg + 1]
                        else:
                            other = q
                        nc.vector.tensor_tensor(
                            lin3(btmp[dc][:])[:, gg:gg + 1], q, other, AX.mult)
                    nc.vector.tensor_tensor(
                        rs[(n, bg)][:, cs], btmp[0][:, cs], btmp[1][:, cs],
                        AX.add)

                ps_sums = {}

                def bil_sums(bg, part="all"):
                    # 5 reductions into one [5,512] PSUM via indicator lhsT;
                    # "pos"/"neg" split lets the pos-side matmuls fire
                    # before the neg products land.
                    if part in ("all", "pos"):
                        ps_sums[bg] = pqpool.tile([128, 512], F32,
                                                  name=f"ps_sums{bg}",
                                                  tag="pt0")
                    ps_s = ps_sums[bg]
                    idx = {"all": (0, 1, 2, 3, 4), "pos": (0, 1, 3),
                           "neg": (2, 4)}[part]
                    for k in idx:
                        nc.tensor.matmul(ps_s[0:5, :],
                                         ind5[:, 5 * k:5 * k + 5],
                                         rs[(SUMS[k], bg)][:],
                                         start=(k == idx[0] and
                                                part in ("all", "pos")),
                                         stop=(k == idx[-1] and
                                               part in ("all", "neg")))
                    if part in ("all", "neg"):
                        ssb = bpool.tile([5, 512], F32, name=f"sums_sb{bg}",
                                         tag="sums_sb", bufs=2)
                        nc.scalar.copy(ssb[:], ps_s[0:5, :])
                        nc.sync.dma_start(
                            sums_out[bg:bg + 1, :].rearrange(
                                "r (p c) -> r p c", p=5),
                            ssb[:])

                # schedule[g] = thunks emitted just before pair g
                NG = 2 * NPAIR
                schedule = [[] for _ in range(NG + 4)]
                schedule[0].append(
                    lambda: nc.gpsimd.dma_start(pam[:], pam_in[:]))
                schedule[0].append(
                    lambda: nc.gpsimd.dma_start(wbt[:], wbt_in[:]))
                schedule[0].append(
                    lambda: nc.gpsimd.dma_start(ind5[:], ind5_in[:]))
                # bdt in 8-block chunks, spread to smooth DMA-bus load
                for c in range(2, 8):
                    schedule[max(0, 4 * c - 12)].append(
                        lambda lo=8 * c: dma_bdt("pos", lo, lo + 8))
                for c in range(8):
                    schedule[4 * c + 18].append(
                        lambda lo=8 * c: dma_bdt("neg", lo, lo + 8))
                for b in range(2):
                    for s in range(NSLAB):
                        g = 32 * b + 2 * s - 16
                        if g >= 0:
                            schedule[g].append(
                                lambda br=branches[b], s=s: _slab(br, s))
                # hoisted bilinear work (deps: pos poolt grp0/1 by g~18,
                # grp2/3 by g~36; neg grp0/1 by g~50). Positions avoid the
                # post-group-boundary pairs where Act does poolt copies.
                schedule[20].append(lambda: bil_ut(0, 0))
                schedule[21].append(lambda: bil_ut(0, 1))
                schedule[22].append(lambda: bil_prod("ssa", 0))
                schedule[23].append(lambda: bil_prod("ssp", 0))
                schedule[24].append(lambda: bil_prod("rwp", 0))
                schedule[36].append(lambda: bil_ut(1, 0))
                schedule[37].append(lambda: bil_ut(1, 1))
                schedule[38].append(lambda: bil_prod("ssa", 1))
                schedule[39].append(lambda: bil_prod("ssp", 1))
                schedule[40].append(lambda: bil_prod("rwp", 1))
                schedule[52].append(lambda: bil_prod("ssn", 0))
                schedule[53].append(lambda: bil_prod("rwn", 0))
                schedule[55].append(lambda: bil_sums(0))
                schedule[61].append(lambda: bil_prod_g("ssn", 1, 0))
                schedule[62].append(lambda: bil_prod_g("rwn", 1, 0))

                # head: minimal serial prefix + PE p-state warmup. ~400
                # tiny matmuls keep PE continuously busy through the DMA
                # head so real matmuls start fully ramped.
                warm = bpool.tile([128, 16], BF16, name="warm", tag="warm")
                nc.vector.memset(warm[:], 0.0)
                one_f = bpool.tile([128, 1], F32, name="one_f", tag="one_f")
                nc.vector.memset(one_f[:], 1.0)
                ps_warm = pqpool.tile([128, 512], F32, name="ps_warm",
                                      tag="pt0")
                for _ in range(400):
                    nc.tensor.matmul(ps_warm[0:16, 0:16], warm[:],
                                     warm[:], start=True, stop=True)
                # dummy reader so the verifier sees ps_warm consumed
                nc.vector.tensor_copy(btmp[0][0:16, 0:16],
                                      ps_warm[0:16, 0:16])
                nc.sync.dma_start(wg[:], wg_in[:])
                _slab("pos", 0)
                _slab("pos", 1)
                dma_bdt("pos", 0, 16)
                nc.sync.dma_start(a_rep[:], a_in[:])
                if use_bias:
                    nc.sync.dma_start(bg_row[:], bgcn_in[:])
                    nc.gpsimd.partition_broadcast(bg_bc[:], bg_row[:])
                for s in range(2, 8):
                    _slab("pos", s)

                state = {}
                ps_pt = {}
                pending_copy = []   # staggered poolt copies

                def stage_xw(g):
                    br = branches[g // NPAIR]
                    B0 = 2 * (g % NPAIR)
                    xt = xt_tiles[(br, B0 // 4)]
                    ps_xw = pspool.tile([128, 2 * DOUT], F32,
                                        name=f"ps_xw{g}", tag="xw")
                    for half in range(2):
                        bb = (B0 + half) % 4
                        for k in range(4):
                            nc.tensor.matmul(
                                ps_xw[:, half * DOUT:(half + 1) * DOUT],
                                xt[:, k * 512 + bb * 128:
                                   k * 512 + (bb + 1) * 128],
                                wg[:, k * DOUT:(k + 1) * DOUT],
                                start=(k == 0), stop=(k == 3))
                    xw_sb = bpool.tile([128, 2 * DOUT], BF16,
                                       name=f"xw_sb{g}", tag="xw_sb", bufs=4)
                    nc.vector.tensor_copy(xw_sb[:], ps_xw[:])
                    state[g] = {"br": br, "B0": B0, "xw_sb": xw_sb}

                def stage_agg(g):
                    st = state[g]
                    br, B0, xw_sb = st["br"], st["B0"], st["xw_sb"]
                    ps_agg = pspool.tile([128, 2 * DOUT], F32,
                                         name=f"ps_agg{g}", tag="agg")
                    for half in range(2):
                        B = B0 + half
                        nc.tensor.matmul(
                            ps_agg[:, half * DOUT:(half + 1) * DOUT],
                            bdt[br][:, B * 128:(B + 1) * 128],
                            xw_sb[:, half * DOUT:(half + 1) * DOUT],
                            start=True, stop=True)
                    h = bpool.tile([128, 2 * DOUT], BF16,
                                   name=f"h{g}", tag="h", bufs=4)
                    if use_bias:
                        t0 = bpool.tile([128, 2 * DOUT], BF16,
                                        name=f"t0_{g}", tag="t0", bufs=4)
                        nc.vector.tensor_tensor(
                            t0[:].rearrange("p (v c) -> p v c", v=2),
                            ps_agg[:].rearrange("p (v c) -> p v c", v=2),
                            bg_bc[:].unsqueeze(1).broadcast_to((128, 2, DOUT)),
                            AX.add)
                        nc.scalar.activation(
                            h[:], t0[:], mybir.ActivationFunctionType.Prelu,
                            alpha=a_rep[:, 0:1])
                    else:
                        nc.scalar.activation(
                            h[:], ps_agg[:],
                            mybir.ActivationFunctionType.Prelu,
                            alpha=a_rep[:, 0:1])
                    st["h"] = h

                def flush_pending():
                    # last group's copies: split across Act/DVE to shorten
                    # the tail dependency chain
                    while pending_copy:
                        br_, grp_, dc_, pt_ = pending_copy.pop(0)
                        dst = poolt[br_][dc_][:, grp_ * 512:(grp_ + 1) * 512]
                        if dc_ == 0:
                            nc.scalar.copy(dst, pt_[dc_][:])
                        else:
                            nc.vector.tensor_copy(dst, pt_[dc_][:])
                            del ps_pt[(br_, grp_)]

                def stage_pool(g):
                    if pending_copy:
                        br_, grp_, dc_, pt_ = pending_copy.pop(0)
                        nc.scalar.copy(
                            poolt[br_][dc_][:, grp_ * 512:(grp_ + 1) * 512],
                            pt_[dc_][:])
                        if dc_ == 1:
                            del ps_pt[(br_, grp_)]
                    st = state.pop(g)
                    br, B0, h = st["br"], st["B0"], st["h"]
                    grp = B0 // 16
                    if (br, grp) not in ps_pt:
                        ps_pt[(br, grp)] = [
                            pqpool.tile([128, 512], F32,
                                        name=f"pt{dc}_{br}{grp}", tag=f"pt{dc}")
                            for dc in range(2)]
                    pt = ps_pt[(br, grp)]
                    for half in range(2):
                        bi = (B0 + half) % 16
                        for dc in range(2):
                            nc.tensor.matmul(
                                pt[dc][:, bi * 32:(bi + 1) * 32],
                                h[:, half * DOUT + dc * 128:
                                  half * DOUT + (dc + 1) * 128],
                                pam[:], start=True, stop=True)
                    if B0 % 16 == 14:
                        pending_copy.append((br, grp, 0, pt))
                        pending_copy.append((br, grp, 1, pt))

                for g in range(NG + 2):
                    if g < len(schedule):
                        for th in schedule[g]:
                            th()
                    if g < NG:
                        stage_xw(g)
                    if 1 <= g <= NG:
                        stage_agg(g - 1)
                    if g >= 2:
                        stage_pool(g - 2)
                flush_pending()
                # ---- tail: bg1 last-group products + reductions + out ----
                bil_sums(1, "pos")
                bil_prod_g("ssn", 1, 1)
                bil_prod_g("rwn", 1, 1)
                bil_sums(1, "neg")

    nc.finalize()
    return nc


def _prep(inputs):
    """Host-side marshalling: shard + layout + dtype prep for the 8 cores."""
    bf = ml_dtypes.bfloat16

    def xt_prep(x):
        xb = np.asarray(x, np.float32).astype(bf).view(np.uint16)
        xb = xb.reshape(N_CORES, NC_NODES, 4, 128).transpose(0, 3, 2, 1)
        return np.ascontiguousarray(xb).reshape(N_CORES, 128, 4 * NC_NODES) \
            .view(bf)

    def bdt_prep(src, dst, w):
        src = np.asarray(src).astype(np.int64)
        dst = np.asarray(dst).astype(np.int64)
        w = np.asarray(w, np.float64)
        sub = src // S
        c = (src % S) * S + (dst % S)
        A = np.bincount(sub * EPB + c, weights=w,
                        minlength=B_TOT * EPB).astype(np.float32)
        A8 = A.reshape(N_CORES, NBLK, 16, S, S)      # [core, B, j, s, d]
        out = np.zeros((N_CORES, NBLK, 16, S, 16, S), np.float32)
        for j in range(16):
            out[:, :, j, :, j, :] = A8[:, :, j]
        out = out.transpose(0, 2, 3, 1, 4, 5).reshape(N_CORES, 128, NBLK * 128)
        return np.ascontiguousarray(out).astype(bf)

    xt_pos = xt_prep(inputs["pos_x"])
    xt_neg = xt_prep(inputs["neg_x"])
    bdt_pos = bdt_prep(inputs["pos_src"], inputs["pos_dst"], inputs["pos_w"])
    bdt_neg = bdt_prep(inputs["neg_src"], inputs["neg_dst"], inputs["neg_w"])

    wg = np.asarray(inputs["W_gcn"], np.float32).astype(bf)
    wg_sb = np.ascontiguousarray(
        wg.reshape(4, 128, DOUT).transpose(1, 0, 2).reshape(128, 4 * DOUT))
    wbt = np.asarray(inputs["W_bil"], np.float32).T.astype(bf)   # [e, d]
    wbt_sb = np.ascontiguousarray(
        wbt.reshape(2, 128, 2, 128).transpose(1, 0, 2, 3).reshape(128, 512))
    pam = np.zeros((128, 32), np.float32)
    for j in range(16):
        pam[S * j:S * j + 7, j] = 1.0 / 7.0
        pam[S * j + 7, 16 + j] = 1.0
    ind5 = np.zeros((5, 5), np.float32)
    np.fill_diagonal(ind5, 1.0)
    ind5 = np.tile(ind5.reshape(1, 25), (128, 1))
    a_rep = np.full((128, 1), float(np.asarray(inputs["prelu_a"])), np.float32)
    bgcn = np.asarray(inputs["b_gcn"], np.float32).reshape(1, DOUT)
    use_bias = bool(np.any(bgcn))

    consts = {
        "wg_in": wg_sb.astype(bf), "wbt_in": wbt_sb.astype(bf),
        "pam_in": pam.astype(bf), "ind5_in": ind5.astype(bf),
        "a_in": a_rep,
    }
    if use_bias:
        consts["bgcn_in"] = bgcn

    in_maps = []
    for k in range(N_CORES):
        m = dict(consts)
        m["xt_pos"] = xt_pos[k]
        m["xt_neg"] = xt_neg[k]
        m["bdt_pos"] = bdt_pos[k]
        m["bdt_neg"] = bdt_neg[k]
        in_maps.append(m)
    return in_maps, use_bias


def kernel(**inputs):
    in_maps, use_bias = _prep(inputs)
    if use_bias not in _KERNEL_CACHE:
        _KERNEL_CACHE[use_bias] = _build(use_bias)
    nc = _KERNEL_CACHE[use_bias]
    res = run_bass_kernel_spmd(nc, in_maps, core_ids=list(range(N_CORES)))
    bbil = float(np.asarray(inputs["b_bil"]).ravel()[0])
    pos_parts, neg_parts = [], []
    for r in res.results:
        s = np.asarray(r["sums_out"], np.float64).reshape(2, 5, 512)
        ssa, ssp, ssn, rwp, rwn = (s[:, i, :] for i in range(5))  # [2, 512]
        na = np.maximum(np.sqrt(ssa), EPS)
        pos = rwp / (np.maximum(np.sqrt(ssp), EPS) * na) + bbil
        neg = rwn / (np.maximum(np.sqrt(ssn), EPS) * na) + bbil
        pos_parts.append(pos.reshape(-1))
        neg_parts.append(neg.reshape(-1))
    pos = np.concatenate(pos_parts).astype(np.float32)
    neg = np.concatenate(neg_parts).astype(np.float32)
    return pos, neg
